# revision 1
# baseline (speedup 1.0000x reference)
"""Majority-vote (binary bincount+argmax) Trainium2 Bass kernel.

Problem: inputs [31, 2_000_000] int32 with values in {0, 1}. For each batch
column, output argmax of the class histogram = 1 iff sum of the 31 votes
>= 16 (31 is odd, so no ties), else 0. Output: [2_000_000] int32.

Strategy: pure data-parallel across 8 NeuronCores — each core gets a
contiguous 250_000-column slice, viewed on-chip as [125 partitions, 2000].
Per core: 31 x 1 MB contiguous DMA loads (one per voter row), a serial DVE
int32 accumulate chain, one tensor_scalar is_ge(16) compare, one 1 MB store.
Memory-bound: ~31 MB read / core.
"""

import numpy as np

V = 31                  # voters
BATCH = 2_000_000
N_CORES = 8
B = BATCH // N_CORES    # 250_000 batch columns per core
P = 125                 # SBUF partitions used (125 * 2000 = 250_000)
Q = B // P              # 2000 free elements per partition
NCH = 2                 # free-dim chunks per core
F = Q // NCH            # chunk free size
VT_BUFS = 40            # voter-tile slots (40 * F * 4B = 160 KB/partition)
THRESH = (V + 1) // 2   # 16

_cache = {}


def _build_nc():
    import concourse.bacc as bacc
    import concourse.mybir as mybir
    from concourse.mybir import AluOpType
    from concourse.tile import TileContext

    nc = bacc.Bacc("TRN2", target_bir_lowering=False, debug=False)
    x = nc.dram_tensor("x", [V, P, Q], mybir.dt.int32, kind="ExternalInput")
    y = nc.dram_tensor("y", [P, Q], mybir.dt.int32, kind="ExternalOutput")

    with TileContext(nc) as tc:
        with tc.tile_pool(name="vt", bufs=VT_BUFS) as vpool, \
             tc.tile_pool(name="acc", bufs=2) as apool, \
             tc.tile_pool(name="ot", bufs=2) as opool:
            for ch in range(NCH):
                sl = slice(ch * F, (ch + 1) * F)
                acc = apool.tile([P, F], mybir.dt.int32)
                t0 = None
                for v in range(V):
                    t = vpool.tile([P, F], mybir.dt.int32)
                    eng = nc.sync if v % 2 == 0 else nc.scalar
                    eng.dma_start(t[:], x[v, :, sl])
                    if v == 0:
                        t0 = t
                    elif v == 1:
                        nc.vector.tensor_tensor(acc[:], t0[:], t[:], AluOpType.add)
                    else:
                        nc.vector.tensor_tensor(acc[:], acc[:], t[:], AluOpType.add)
                ot = opool.tile([P, F], mybir.dt.int32)
                nc.vector.tensor_scalar(ot[:], acc[:], THRESH, None, AluOpType.is_ge)
                eng = nc.sync if ch % 2 == 0 else nc.scalar
                eng.dma_start(y[:, sl], ot[:])
    nc.compile()
    return nc


def _get_nc():
    if "nc" not in _cache:
        _cache["nc"] = _build_nc()
    return _cache["nc"]


def _run(in_maps, **kwargs):
    from concourse.bass_utils import run_bass_kernel_spmd

    return run_bass_kernel_spmd(
        _get_nc(), in_maps, core_ids=list(range(N_CORES)), **kwargs
    )


def _shard(inputs):
    in_maps = []
    for i in range(N_CORES):
        xi = np.ascontiguousarray(inputs[:, i * B:(i + 1) * B]).reshape(V, P, Q)
        in_maps.append({"x": xi})
    return in_maps


def _gather(results):
    out = np.empty(BATCH, dtype=np.int32)
    for i in range(N_CORES):
        out[i * B:(i + 1) * B] = results[i]["y"].reshape(B)
    return out


def kernel(inputs: np.ndarray) -> np.ndarray:
    inputs = np.asarray(inputs)
    assert inputs.shape == (V, BATCH), inputs.shape
    inputs = inputs.astype(np.int32, copy=False)
    res = _run(_shard(inputs))
    return _gather(res.results)



# revision 2
# speedup vs baseline: 1.4146x; 1.4146x over previous
"""Majority-vote (binary bincount+argmax) Trainium2 Bass kernel.

Problem: inputs [31, 2_000_000] int32 with values in {0, 1}. For each batch
column, output argmax of the class histogram = 1 iff sum of the 31 votes
>= 16 (31 is odd, so no ties), else 0. Output: [2_000_000] int32.

Strategy: pure data-parallel across 8 NeuronCores — each core gets a
contiguous 250_000-column slice, viewed on-chip as [125 partitions, 2000].
Per core: 31 x 1 MB loads issued on nc.gpsimd (SWDGE sprays each DMA's
descriptors across all 16 SDMA engines ~341 GB/s, vs ~131 GB/s for the
HWDGE dynamic queues), a serial DVE int32 accumulate chain, one
tensor_scalar is_ge(16) compare, one 1 MB store on nc.sync.
Memory-bound: ~31 MB read / core.
"""

import numpy as np

V = 31                  # voters
BATCH = 2_000_000
N_CORES = 8
B = BATCH // N_CORES    # 250_000 batch columns per core
P = 125                 # SBUF partitions used (125 * 2000 = 250_000)
Q = B // P              # 2000 free elements per partition
VT_BUFS = 12            # voter-tile slots (12 * 8000B = 96 KB/partition)
THRESH = (V + 1) // 2   # 16

_cache = {}


def _build_nc():
    import concourse.bacc as bacc
    import concourse.mybir as mybir
    from concourse.mybir import AluOpType
    from concourse.tile import TileContext

    nc = bacc.Bacc("TRN2", target_bir_lowering=False, debug=False)
    x = nc.dram_tensor("x", [V, P, Q], mybir.dt.int32, kind="ExternalInput")
    y = nc.dram_tensor("y", [P, Q], mybir.dt.int32, kind="ExternalOutput")

    with TileContext(nc) as tc:
        with tc.tile_pool(name="vt", bufs=VT_BUFS) as vpool, \
             tc.tile_pool(name="acc", bufs=1) as apool, \
             tc.tile_pool(name="ot", bufs=1) as opool:
            acc = apool.tile([P, Q], mybir.dt.int32)
            t0 = None
            for v in range(V):
                t = vpool.tile([P, Q], mybir.dt.int32)
                nc.gpsimd.dma_start(t[:], x[v, :, :])
                if v == 0:
                    t0 = t
                elif v == 1:
                    nc.vector.tensor_tensor(acc[:], t0[:], t[:], AluOpType.add)
                else:
                    nc.vector.tensor_tensor(acc[:], acc[:], t[:], AluOpType.add)
            ot = opool.tile([P, Q], mybir.dt.int32)
            nc.vector.tensor_scalar(ot[:], acc[:], THRESH, None, AluOpType.is_ge)
            nc.sync.dma_start(y[:, :], ot[:])
    nc.compile()
    return nc


def _get_nc():
    if "nc" not in _cache:
        _cache["nc"] = _build_nc()
    return _cache["nc"]


def _run(in_maps, **kwargs):
    from concourse.bass_utils import run_bass_kernel_spmd

    return run_bass_kernel_spmd(
        _get_nc(), in_maps, core_ids=list(range(N_CORES)), **kwargs
    )


def _shard(inputs):
    in_maps = []
    for i in range(N_CORES):
        xi = np.ascontiguousarray(inputs[:, i * B:(i + 1) * B]).reshape(V, P, Q)
        in_maps.append({"x": xi})
    return in_maps


def _gather(results):
    out = np.empty(BATCH, dtype=np.int32)
    for i in range(N_CORES):
        out[i * B:(i + 1) * B] = results[i]["y"].reshape(B)
    return out


def kernel(inputs: np.ndarray) -> np.ndarray:
    inputs = np.asarray(inputs)
    assert inputs.shape == (V, BATCH), inputs.shape
    inputs = inputs.astype(np.int32, copy=False)
    res = _run(_shard(inputs))
    return _gather(res.results)
